# revision 35
# baseline (speedup 1.0000x reference)
"""Trainium2 Bass kernel for nn_Evolution_4664334483942 (moe_routing).

Model: per-token relation-specific linear (MoE dispatch) feeding a packed
variable-length-sequence LSTM.

Strategy (data-parallel over sequences, 8 cores, no collectives):
  - Global batch b (0..1023) assigned to core b % 8.  Every core then holds
    128 sequences with lengths 128,127,...,1 (identical structure on every
    core), 8256 tokens each.
  - Host folds W_ih @ W_rel[r].T into per-relation fused weights so the MoE
    projection and the LSTM input projection collapse into ONE GEMM:
        gx[n] = x[n] @ Wfuse[rel_n].T + (W_ih b_rel[rel_n] + b_ih + b_hh)
  - All matmul operands are bf16 (stationary bf16 enables the PE fast
    weight load; moving bf16 allows 1024-wide streams spanning 2 PSUM
    banks), accumulation stays f32 in PSUM, the LSTM cell state stays f32.
  - Phase 1 (device): dense bf16 GEMM over rel-sorted 128-token tiles
    (per-rel tile counts sized to the worst core), writing gx (bf16) to
    DRAM.
  - Phase 2 (device): 128 sequential LSTM steps.  Each step gathers its
    gx rows via indirect DMA (per-core index table = data, so the SPMD
    instruction stream stays core-independent), injects them into the two
    1024-wide gate PSUM tiles via identity matmuls, accumulates h @ W_hh.T
    on top, applies sigmoid/tanh on ScalarE, c/h updates on VectorE,
    PE-transposes h (bf16) for the next step, and streams h out to DRAM.
"""

import numpy as np
import ml_dtypes

import concourse.bass as bass
import concourse.mybir as mybir
import concourse.tile as tile
from concourse import bass_utils
from concourse.masks import make_identity
from bass_rust import add_dep_helper
from concourse.vector_clock import ScopedClock

F32 = mybir.dt.float32
BF16 = mybir.dt.bfloat16
I32 = mybir.dt.int32
AF = mybir.ActivationFunctionType
BF16NP = ml_dtypes.bfloat16

NCORES = 8

# Problem constants (hardcoded; kernel.py must be self-contained).
D = 512          # hidden dim
R = 8            # relations
T = 128          # max sequence length / LSTM steps
B = 1024         # global sequences
KD = D // 128    # contraction k-tiles
G = 4 * D        # gate width (2048)
HW = 1024        # matmul moving-stream width (2 PSUM banks)

# Results of the last device run (test harness reads exec_time_ns from here).
LAST_RESULTS = None


# ---------------------------------------------------------------------------
# Walrus in this toolchain accepts only ONE sync-wait command per instruction;
# Tile's wait assignment can attach several.  Peel the extras onto same-engine
# NOPs placed immediately before the offending instruction.
# ---------------------------------------------------------------------------
def _split_waits_in_list(nc, insts, max_waits=1):
    out = []
    for inst in insts:
        si = inst.sync_info
        if si is not None and si.on_wait is not None and len(si.on_wait) > max_waits:
            waits = list(si.on_wait)
            for w in waits[max_waits:]:
                nop = mybir.InstNoOp(
                    name=nc.get_next_instruction_name(), ins=[], outs=[],
                )
                nop.engine = inst.engine
                nop.sync_info = mybir.SyncInfo(on_wait=[w], on_update=[])
                out.append(nop)
            inst.sync_info = mybir.SyncInfo(
                on_wait=waits[:max_waits], on_update=list(si.on_update or [])
            )
        out.append(inst)
    return out


class PatchedTileContext(tile.TileContext):
    def _lower_ordered_insts(self, ordered):
        for bb_name in list(ordered.keys()):
            ordered[bb_name] = _split_waits_in_list(self.nc, ordered[bb_name])
        super()._lower_ordered_insts(ordered)

    def _drain_and_barrier(self, tick_clock, wait_clock):
        nop_inst = self.nc.sync.nop()
        wait_clock.add_sem_waits(
            nop_inst.ins, ScopedClock({None: tick_clock.global_clock})
        )
        si = nop_inst.ins.sync_info
        if si is not None and si.on_wait and len(si.on_wait) > 1:
            waits = list(si.on_wait)
            nop_inst.ins.sync_info = mybir.SyncInfo(
                on_wait=[waits[0]], on_update=list(si.on_update or [])
            )
            for w in waits[1:]:
                extra = self.nc.sync.nop()
                extra.ins.sync_info = mybir.SyncInfo(on_wait=[w], on_update=[])
        self.nc.sync.drain()
        self.nc.all_engine_barrier()
        assert self.sems is not None
        popped = self.nc._tile_sem_poison_stack.pop()
        assert popped is self._sem_poison
        self.nc.clear_and_free_semaphores(list(self.sems.allocated().values()))
        self.nc.all_engine_barrier()


# ---------------------------------------------------------------------------
# Device program (core-independent instruction stream; per-core variation is
# carried entirely by input data: xt tile contents and the gather index table)
#
# ntc: tuple of R ints — tiles per relation (same on every core).
# emit_order: phase-1 tile emission order (physical tile indices), sorted by
#   gather deadline so tiles interleave into the LSTM's tensor-engine gaps.
# K: K[t] = number of tiles (prefix of emit_order) whose gx rows must be
#   written before the step-t gather may run (worst core).
# ---------------------------------------------------------------------------
def build_program(ntc, emit_order, K, nsteps=T):
    ntiles = sum(ntc)
    nrows = ntiles * 128
    nloc = nsteps * (nsteps + 1) // 2

    # physical tile order: rel-major
    tile_rel = []
    for r in range(R):
        tile_rel.extend([r] * ntc[r])

    nc = bass.Bass(target_bir_lowering=False, debug=False, trn_type="TRN2")

    xt = nc.dram_tensor("xt", [ntiles, 128, KD, 128], BF16, kind="ExternalInput").ap()
    wf = nc.dram_tensor("wf", [R, 128, KD, G], BF16, kind="ExternalInput").ap()
    wh = nc.dram_tensor("wh", [128, KD, G], BF16, kind="ExternalInput").ap()
    brep = nc.dram_tensor("brep", [R, 128, G], BF16, kind="ExternalInput").ap()
    gidx = nc.dram_tensor("gidx", [128, nsteps], I32, kind="ExternalInput").ap()
    out = nc.dram_tensor("out", [nloc, D], BF16, kind="ExternalOutput").ap()
    gx = nc.dram_tensor("gx", [nrows, G], BF16).ap()

    loc_bs = [nsteps - t for t in range(nsteps)]
    loc_off = np.concatenate([[0], np.cumsum(loc_bs)]).astype(int)

    with PatchedTileContext(nc) as tc:
        with tc.tile_pool(name="p1_xt", bufs=2) as xt_pool, \
             tc.tile_pool(name="p1_wf", bufs=8) as wf_pool, \
             tc.tile_pool(name="p1_bi", bufs=8) as bi_pool, \
             tc.tile_pool(name="p1_gx", bufs=1) as gxs_pool, \
             tc.tile_pool(name="p2_const", bufs=1) as const_pool, \
             tc.tile_pool(name="p2_gx", bufs=2) as gx_pool, \
             tc.tile_pool(name="p2_act", bufs=1) as act_pool, \
             tc.tile_pool(name="p2_st", bufs=1) as st_pool, \
             tc.tile_pool(name="p2_h", bufs=2) as h_pool, \
             tc.tile_pool(name="p2_ht", bufs=2) as ht_pool, \
             tc.tile_pool(name="p2_ps", bufs=4, space="PSUM") as ps_pool, \
             tc.tile_pool(name="p1_ps", bufs=1, space="PSUM") as p1ps_pool, \
             tc.tile_pool(name="p2_tr", bufs=1, space="PSUM") as tr_pool:

            # ---------------- phase-1 weights: all rels resident, loaded
            # lazily (first tile of each rel triggers the load) so prologue
            # tiles don't queue behind 16MB of weight DMA ------------------
            wf_sbs, bi_sbs = {}, {}

            def ensure_wf(r):
                if r not in wf_sbs:
                    wf_sb = wf_pool.tile([128, KD, G], BF16, tag="wf_sb")
                    nc.sync.dma_start(wf_sb[:], wf[r])
                    bi_sb = bi_pool.tile([128, G], BF16, tag="bi_sb")
                    nc.sync.dma_start(bi_sb[:], brep[r])
                    wf_sbs[r] = wf_sb
                    bi_sbs[r] = bi_sb

            # ---------------- phase-1 part emitter -----------------------
            # a part = one 1024-wide gate half of one tile (8 matmuls + add)
            p1_writes = []          # one DMA-write instr per tile, emit order
            p1_tile_state = {}
            emit_pos = [0]          # next part index (2 parts per tile)

            def emit_p1_part():
                pi = emit_pos[0]
                emit_pos[0] += 1
                e, jb = pi // 2, pi % 2
                i = emit_order[e]
                r = tile_rel[i]
                ensure_wf(r)
                if jb == 0:
                    xt_sb = xt_pool.tile([128, KD, 128], BF16, tag="xt_sb")
                    nc.sync.dma_start(xt_sb[:], xt[i])
                    gxs = gxs_pool.tile([128, G], BF16, tag="gxs")
                    p1_tile_state[e] = (xt_sb, gxs)
                xt_sb, gxs = p1_tile_state[e]
                sl = slice(jb * HW, (jb + 1) * HW)
                ps = p1ps_pool.tile([128, HW], F32, tag="p1ps")
                for k in range(KD):
                    for half in range(2):
                        hs = slice(half * 512, (half + 1) * 512)
                        ws = slice(jb * HW + half * 512,
                                   jb * HW + (half + 1) * 512)
                        nc.tensor.matmul(
                            ps[:, hs], xt_sb[:, k, :], wf_sbs[r][:, k, ws],
                            start=(k == 0), stop=(k == KD - 1),
                        )
                # bias add + cast to bf16 in one DVE pass
                nc.vector.tensor_add(gxs[:, sl], ps[:], bi_sbs[r][:, sl])
                if jb == 1:
                    wi = nc.sync.dma_start(gx[i * 128:(i + 1) * 128, :], gxs[:])
                    p1_writes.append(wi.ins)
                    del p1_tile_state[e]

            def ensure_written(n):
                while len(p1_writes) < min(n, ntiles):
                    emit_p1_part()

            def fill_to(n):
                target = 2 * min(n, ntiles)
                if emit_pos[0] < target:
                    emit_p1_part()

            # ---------------- phase 2: LSTM ------------------------------
            wh_sb = const_pool.tile([128, KD, G], BF16)
            nc.sync.dma_start(wh_sb[:], wh[:])
            idx_sb = const_pool.tile([128, nsteps], I32)
            nc.sync.dma_start(idx_sb[:], gidx[:])
            ident_b = const_pool.tile([128, 128], BF16)
            make_identity(nc, ident_b[:])

            c_sb = st_pool.tile([128, D], F32)
            tmp1 = st_pool.tile([128, D], F32)

            ht_sb = None
            gxt_tiles = {}
            banks = {}   # (t, jb) -> psum tile [128, 512]; jb = i,f,g,o

            def emit_gather(t):
                ensure_written(K[t])
                gxt = gx_pool.tile([128, G], BF16, tag="gxt")
                gi = nc.gpsimd.indirect_dma_start(
                    out=gxt[:],
                    out_offset=None,
                    in_=gx[0:nrows, :],
                    in_offset=bass.IndirectOffsetOnAxis(
                        ap=idx_sb[:, t:t + 1], axis=0
                    ),
                )
                # the tracker cannot see through the dynamic row offsets, so
                # order the gather after the writes it needs explicitly
                for w in p1_writes[:K[t]]:
                    add_dep_helper(gi.ins, w, reason="gather waits gx writes")
                gxt_tiles[t] = gxt

            def emit_ident(t, jb):
                # first write of gate bank jb for step t: gates <- gx rows
                psb = ps_pool.tile([128, 512], F32, tag="ps")
                nc.tensor.matmul(
                    psb[:], ident_b[:],
                    gxt_tiles[t][:, jb * 512:(jb + 1) * 512],
                    start=True, stop=(t == 0),
                )
                banks[(t, jb)] = psb

            # gate bank order: g first so the c-chain starts earliest
            BORD = (2, 0, 1, 3)   # g, i, f, o
            emit_gather(0)
            emit_gather(1)
            for jb in BORD:
                emit_ident(0, jb)
            for t in range(nsteps):
                bs = nsteps - t
                if t + 2 < nsteps:
                    emit_gather(t + 2)
                sif = act_pool.tile([128, 2 * D], F32, tag="sif")
                tg = act_pool.tile([128, D], F32, tag="tg")
                so = act_pool.tile([128, D], BF16, tag="so")

                def do_act_half(jb, hh, pop=False):
                    # half-granularity acts: the c-chain starts after only the
                    # first halves of sigmoid(g/i/f), not the full 512 columns
                    psb = banks.pop((t, jb)) if pop else banks[(t, jb)]
                    dst = {2: tg[:], 0: sif[:, 0:D], 1: sif[:, D:2 * D],
                           3: so[:]}[jb]
                    fn = AF.Tanh if jb == 2 else AF.Sigmoid
                    hs = slice(hh * (D // 2), (hh + 1) * (D // 2))
                    nc.scalar.activation(dst[:, hs], psb[:, hs], fn)

                # recurrent accumulation: consume hT half-by-half (k 0,1 then
                # 2,3) so it pipelines with the previous step's tail; within
                # each half k is outermost so consecutive matmuls share the
                # stationary operand.  Acts fire per bank after its k=3, but
                # the t+1 idents are deferred past the whole block so the k=3
                # tail is not serialized on the activations.
                if t > 0:
                    for ks in ((0, 1), (2, 3)):
                        for k in ks:
                            for jb in BORD:
                                nc.tensor.matmul(
                                    banks[(t, jb)][:],
                                    ht_sb[:, k * 128:(k + 1) * 128],
                                    wh_sb[:, k, jb * 512:(jb + 1) * 512],
                                    start=False,
                                    stop=(k == KD - 1),
                                )
                                if k == KD - 1:
                                    do_act_half(jb, 0)
                    for jb in BORD:
                        do_act_half(jb, 1, pop=True)
                    fill_to(K[min(t + 8, nsteps - 1)])
                else:
                    for jb in BORD:
                        do_act_half(jb, 0)
                        do_act_half(jb, 1, pop=True)
                if t + 1 < nsteps:
                    for jb in BORD:
                        emit_ident(t + 1, jb)
                fill_to(K[min(t + 8, nsteps - 1)])

                # c / h tail at half granularity so the next step's first
                # recurrent matmuls start as soon as half 0 is through
                h_sb = h_pool.tile([128, D], BF16, tag="h_sb")
                if t < nsteps - 1:
                    trp = tr_pool.tile([128, D], BF16, tag="trp")
                    new_ht = ht_pool.tile([128, D], BF16, tag="ht_sb")
                tc_sb = act_pool.tile([128, D], BF16, tag="tc_sb")
                H = D // 2
                for hh in range(2):
                    sl = slice(hh * H, (hh + 1) * H)
                    if t == 0:
                        nc.vector.tensor_tensor(
                            c_sb[:, sl], sif[:, sl], tg[:, sl],
                            mybir.AluOpType.mult,
                        )
                    else:
                        # i*g in place over the sigmoid(i) slice
                        nc.vector.tensor_tensor(
                            sif[:, sl], sif[:, sl], tg[:, sl],
                            mybir.AluOpType.mult,
                        )
                        nc.vector.tensor_tensor(
                            tmp1[:, sl], sif[:, D + hh * H:D + (hh + 1) * H],
                            c_sb[:, sl], mybir.AluOpType.mult,
                        )
                        nc.vector.tensor_add(c_sb[:, sl], tmp1[:, sl],
                                             sif[:, sl])
                    nc.scalar.activation(tc_sb[:, sl], c_sb[:, sl], AF.Tanh)
                    nc.vector.tensor_tensor(
                        h_sb[:, sl], so[:, sl], tc_sb[:, sl],
                        mybir.AluOpType.mult,
                    )
                    if t < nsteps - 1:
                        for k in (2 * hh, 2 * hh + 1):
                            nc.tensor.transpose(
                                trp[:, k * 128:(k + 1) * 128],
                                h_sb[:, k * 128:(k + 1) * 128],
                                ident_b[:],
                            )
                        nc.vector.tensor_copy(new_ht[:, sl], trp[:, sl])
                if t < nsteps - 1:
                    ht_sb = new_ht
                # stream out this step's hidden states (packed rows)
                nc.sync.dma_start(
                    out[int(loc_off[t]):int(loc_off[t]) + bs, :], h_sb[:bs, :]
                )
                # end-of-step fill with a deeper lookahead: covers the
                # h-chain boundary wait so the PE clock gate stays hot
                fill_to(K[min(t + 14, nsteps - 1)])
            ensure_written(ntiles)
    return nc


# ---------------------------------------------------------------------------
# Host-side data marshaling
# ---------------------------------------------------------------------------
def _expected_layout():
    lengths = T - np.arange(B) // NCORES
    batch_sizes = np.array([(lengths > t).sum() for t in range(T)], dtype=np.int32)
    time_idx = np.concatenate(
        [np.full(bs, t, np.int32) for t, bs in enumerate(batch_sizes)]
    )
    batch_idx = np.concatenate(
        [np.arange(bs, dtype=np.int32) for bs in batch_sizes]
    )
    return batch_sizes, time_idx, batch_idx


def _numpy_reference(embed, W_rel, b_rel, W_ih, W_hh, b_ih, b_hh,
                     nodes, rels, time_idx, batch_idx, batch_sizes):
    """Pure-numpy fallback (only used if the packed layout differs from the
    hardcoded one)."""
    n_steps = int(batch_sizes.shape[0])
    max_bs = int(batch_sizes.max())
    x = embed[nodes]
    y = np.zeros_like(x)
    for r in range(W_rel.shape[0]):
        m = rels == r
        y[m] = x[m] @ W_rel[r].T + b_rel[r]
    d = x.shape[-1]
    xp = np.zeros((n_steps, max_bs, d), x.dtype)
    mask = np.zeros((n_steps, max_bs), bool)
    xp[time_idx, batch_idx] = y
    mask[time_idx, batch_idx] = True
    bias = b_ih + b_hh

    def sig(v):
        return 1.0 / (1.0 + np.exp(-v))

    h = np.zeros((max_bs, d), x.dtype)
    c = np.zeros((max_bs, d), x.dtype)
    hs = np.zeros((n_steps, max_bs, d), x.dtype)
    for t in range(n_steps):
        gates = xp[t] @ W_ih.T + h @ W_hh.T + bias
        i, f, g, o = np.split(gates, 4, axis=-1)
        c_new = sig(f) * c + sig(i) * np.tanh(g)
        h_new = sig(o) * np.tanh(c_new)
        m = mask[t][:, None]
        h = np.where(m, h_new, h)
        c = np.where(m, c_new, c)
        hs[t] = h
    return hs[time_idx, batch_idx]


def _prepare_host(inputs, nsteps=T):
    """Build per-core device input dicts + the output unshard map."""
    embed = np.asarray(inputs["embed"], np.float32)
    W_rel = np.asarray(inputs["W_rel"], np.float32)
    b_rel = np.asarray(inputs["b_rel"], np.float32)
    W_ih = np.asarray(inputs["W_ih"], np.float32)
    W_hh = np.asarray(inputs["W_hh"], np.float32)
    b_ih = np.asarray(inputs["b_ih"], np.float32)
    b_hh = np.asarray(inputs["b_hh"], np.float32)
    nodes = np.asarray(inputs["nodes"])
    rels = np.asarray(inputs["rels"])

    nloc = nsteps * (nsteps + 1) // 2

    # fused weights & biases (float64 for accuracy, cast to bf16/f32)
    Wfuse = (W_ih.astype(np.float64) @ W_rel.astype(np.float64))
    Wfuse = Wfuse.astype(np.float32)            # [R, G, D]
    btot = (W_ih.astype(np.float64) @ b_rel.astype(np.float64).T).T \
        + (b_ih + b_hh).astype(np.float64)      # [R, G]
    btot = btot.astype(np.float32)

    wf_host = np.ascontiguousarray(
        Wfuse.transpose(0, 2, 1).reshape(R, KD, 128, G).transpose(0, 2, 1, 3)
    ).astype(BF16NP)                             # [R, 128(dk), KD, G]
    wh_host = np.ascontiguousarray(
        W_hh.T.reshape(KD, 128, G).transpose(1, 0, 2)
    ).astype(BF16NP)                             # [128(dk), KD, G]
    brep_host = np.ascontiguousarray(
        np.broadcast_to(btot[:, None, :], (R, 128, G))
    ).astype(BF16NP)

    # local token enumeration (identical structure for every core)
    t_arr = np.concatenate(
        [np.full(nsteps - t, t, np.int64) for t in range(nsteps)]
    )
    j_arr = np.concatenate(
        [np.arange(nsteps - t, dtype=np.int64) for t in range(nsteps)]
    )
    gbs = NCORES * (nsteps - np.arange(nsteps, dtype=np.int64))
    goff = np.concatenate([[0], np.cumsum(gbs)])

    # per-core per-rel token counts -> shared per-rel tile counts
    core_rel = []
    for core in range(NCORES):
        grow = goff[t_arr] + NCORES * j_arr + core
        rel_loc = rels[grow].astype(np.int64)
        core_rel.append(rel_loc)
    counts = np.array([
        np.bincount(core_rel[core], minlength=R) for core in range(NCORES)
    ])                                           # [NCORES, R]
    ntc = tuple(int(-(-counts[:, r].max() // 128)) for r in range(R))
    ntiles = sum(ntc)
    seg_base = np.concatenate([[0], np.cumsum(ntc)]) * 128  # per-rel row base

    # gather deadlines: n_r(t) = tiles of rel r needed by the step-t gather
    # (worst core); K[t] = total needed tiles; emit_order sorted by deadline.
    # max_cum[r, t] = max over cores of #{tokens of rel r with time <= t}
    max_cum = np.zeros((R, nsteps), np.int64)
    for core in range(NCORES):
        for r in range(R):
            sel = core_rel[core] == r
            cnt_t = np.bincount(t_arr[sel], minlength=nsteps)
            max_cum[r] = np.maximum(max_cum[r], np.cumsum(cnt_t))
    n_rt = -(-max_cum // 128)                    # [R, nsteps]
    K = n_rt.sum(axis=0).astype(int)             # [nsteps]
    tile_base = np.concatenate([[0], np.cumsum(ntc)])
    dl_list = []
    for r in range(R):
        for j in range(ntc[r]):
            need = np.nonzero(n_rt[r] > j)[0]
            dl = int(need[0]) if len(need) else nsteps - 1
            dl_list.append((dl, j, r, int(tile_base[r] + j)))
    dl_list.sort()
    emit_order = [phys for (_, _, _, phys) in dl_list]

    in_maps = []
    for core in range(NCORES):
        grow = goff[t_arr] + NCORES * j_arr + core
        node_loc = nodes[grow]
        rel_loc = core_rel[core]

        order = np.lexsort((j_arr, t_arr, rel_loc))
        # position within each rel segment
        cnt = np.bincount(rel_loc, minlength=R)
        q = np.concatenate([np.arange(c) for c in cnt])
        base_sorted = seg_base[rel_loc[order]]
        prow = np.empty(nloc, np.int64)
        prow[order] = base_sorted + q

        gidx_host = np.zeros((128, nsteps), np.int32)
        gidx_host[j_arr, t_arr] = prow

        Xp = np.zeros((ntiles * 128, D), np.float32)
        Xp[prow] = embed[node_loc]
        xt_host = np.ascontiguousarray(
            Xp.reshape(ntiles, 128, KD, 128).transpose(0, 3, 2, 1)
        ).astype(BF16NP)                         # [NT, 128(dk), KD, 128(tok)]

        in_maps.append({
            "xt": xt_host,
            "wf": wf_host,
            "wh": wh_host,
            "brep": brep_host,
            "gidx": gidx_host,
        })

    unshard = {
        "t_arr": t_arr, "j_arr": j_arr, "goff": goff,
        "nloc": nloc, "ntc": ntc, "emit_order": emit_order, "K": K,
    }
    return in_maps, unshard


def kernel(**inputs):
    global LAST_RESULTS
    import os

    # Verify the packed layout matches the hardcoded structure.
    bs_exp, ti_exp, bi_exp = _expected_layout()
    ok = (
        np.array_equal(np.asarray(inputs["batch_sizes"]), bs_exp)
        and np.array_equal(np.asarray(inputs["time_idx"]), ti_exp)
        and np.array_equal(np.asarray(inputs["batch_idx"]), bi_exp)
        and np.asarray(inputs["embed"]).shape == (50000, D)
    )
    if not ok:
        return _numpy_reference(**{k: np.asarray(v) for k, v in inputs.items()})

    in_maps, unshard = _prepare_host(inputs)

    nc = build_program(unshard["ntc"], unshard["emit_order"], unshard["K"])
    trace = bool(os.environ.get("KERNEL_TRACE"))
    res = bass_utils.run_bass_kernel_spmd(
        nc, in_maps, core_ids=list(range(NCORES)), trace=trace,
    )
    LAST_RESULTS = res

    t_arr = unshard["t_arr"]
    j_arr = unshard["j_arr"]
    goff = unshard["goff"]
    out_full = np.zeros((len(np.asarray(inputs["time_idx"])), D), np.float32)
    for core in range(NCORES):
        grow = goff[t_arr] + NCORES * j_arr + core
        out_full[grow] = np.asarray(res.results[core]["out"], np.float32)
    return out_full


# revision 36
# speedup vs baseline: 1.0163x; 1.0163x over previous
"""Trainium2 Bass kernel for nn_Evolution_4664334483942 (moe_routing).

Model: per-token relation-specific linear (MoE dispatch) feeding a packed
variable-length-sequence LSTM.

Strategy (data-parallel over sequences, 8 cores, no collectives):
  - Global batch b (0..1023) assigned to core b % 8.  Every core then holds
    128 sequences with lengths 128,127,...,1 (identical structure on every
    core), 8256 tokens each.
  - Host folds W_ih @ W_rel[r].T into per-relation fused weights so the MoE
    projection and the LSTM input projection collapse into ONE GEMM:
        gx[n] = x[n] @ Wfuse[rel_n].T + (W_ih b_rel[rel_n] + b_ih + b_hh)
  - All matmul operands are bf16 (stationary bf16 enables the PE fast
    weight load; moving bf16 allows 1024-wide streams spanning 2 PSUM
    banks), accumulation stays f32 in PSUM, the LSTM cell state stays f32.
  - Phase 1 (device): dense bf16 GEMM over rel-sorted 128-token tiles
    (per-rel tile counts sized to the worst core), writing gx (bf16) to
    DRAM.
  - Phase 2 (device): 128 sequential LSTM steps.  Each step gathers its
    gx rows via indirect DMA (per-core index table = data, so the SPMD
    instruction stream stays core-independent), injects them into the two
    1024-wide gate PSUM tiles via identity matmuls, accumulates h @ W_hh.T
    on top, applies sigmoid/tanh on ScalarE, c/h updates on VectorE,
    PE-transposes h (bf16) for the next step, and streams h out to DRAM.
"""

import numpy as np
import ml_dtypes

import concourse.bass as bass
import concourse.mybir as mybir
import concourse.tile as tile
from concourse import bass_utils
from concourse.masks import make_identity
from bass_rust import add_dep_helper
from concourse.vector_clock import ScopedClock

F32 = mybir.dt.float32
BF16 = mybir.dt.bfloat16
I32 = mybir.dt.int32
AF = mybir.ActivationFunctionType
BF16NP = ml_dtypes.bfloat16

NCORES = 8

# Problem constants (hardcoded; kernel.py must be self-contained).
D = 512          # hidden dim
R = 8            # relations
T = 128          # max sequence length / LSTM steps
B = 1024         # global sequences
KD = D // 128    # contraction k-tiles
G = 4 * D        # gate width (2048)
HW = 1024        # matmul moving-stream width (2 PSUM banks)

# Results of the last device run (test harness reads exec_time_ns from here).
LAST_RESULTS = None


# ---------------------------------------------------------------------------
# Walrus in this toolchain accepts only ONE sync-wait command per instruction;
# Tile's wait assignment can attach several.  Peel the extras onto same-engine
# NOPs placed immediately before the offending instruction.
# ---------------------------------------------------------------------------
def _split_waits_in_list(nc, insts, max_waits=1):
    out = []
    for inst in insts:
        si = inst.sync_info
        if si is not None and si.on_wait is not None and len(si.on_wait) > max_waits:
            waits = list(si.on_wait)
            for w in waits[max_waits:]:
                nop = mybir.InstNoOp(
                    name=nc.get_next_instruction_name(), ins=[], outs=[],
                )
                nop.engine = inst.engine
                nop.sync_info = mybir.SyncInfo(on_wait=[w], on_update=[])
                out.append(nop)
            inst.sync_info = mybir.SyncInfo(
                on_wait=waits[:max_waits], on_update=list(si.on_update or [])
            )
        out.append(inst)
    return out


class PatchedTileContext(tile.TileContext):
    def _lower_ordered_insts(self, ordered):
        for bb_name in list(ordered.keys()):
            ordered[bb_name] = _split_waits_in_list(self.nc, ordered[bb_name])
        super()._lower_ordered_insts(ordered)

    def _drain_and_barrier(self, tick_clock, wait_clock):
        nop_inst = self.nc.sync.nop()
        wait_clock.add_sem_waits(
            nop_inst.ins, ScopedClock({None: tick_clock.global_clock})
        )
        si = nop_inst.ins.sync_info
        if si is not None and si.on_wait and len(si.on_wait) > 1:
            waits = list(si.on_wait)
            nop_inst.ins.sync_info = mybir.SyncInfo(
                on_wait=[waits[0]], on_update=list(si.on_update or [])
            )
            for w in waits[1:]:
                extra = self.nc.sync.nop()
                extra.ins.sync_info = mybir.SyncInfo(on_wait=[w], on_update=[])
        self.nc.sync.drain()
        self.nc.all_engine_barrier()
        assert self.sems is not None
        popped = self.nc._tile_sem_poison_stack.pop()
        assert popped is self._sem_poison
        self.nc.clear_and_free_semaphores(list(self.sems.allocated().values()))
        self.nc.all_engine_barrier()


# ---------------------------------------------------------------------------
# Device program (core-independent instruction stream; per-core variation is
# carried entirely by input data: xt tile contents and the gather index table)
#
# ntc: tuple of R ints — tiles per relation (same on every core).
# emit_order: phase-1 tile emission order (physical tile indices), sorted by
#   gather deadline so tiles interleave into the LSTM's tensor-engine gaps.
# K: K[t] = number of tiles (prefix of emit_order) whose gx rows must be
#   written before the step-t gather may run (worst core).
# ---------------------------------------------------------------------------
def build_program(ntc, emit_order, K, nsteps=T):
    ntiles = sum(ntc)
    nrows = ntiles * 128
    nloc = nsteps * (nsteps + 1) // 2

    # physical tile order: rel-major
    tile_rel = []
    for r in range(R):
        tile_rel.extend([r] * ntc[r])

    nc = bass.Bass(target_bir_lowering=False, debug=False, trn_type="TRN2")

    xt = nc.dram_tensor("xt", [ntiles, 128, KD, 128], BF16, kind="ExternalInput").ap()
    wf = nc.dram_tensor("wf", [R, 128, KD, G], BF16, kind="ExternalInput").ap()
    wh = nc.dram_tensor("wh", [128, KD, G], BF16, kind="ExternalInput").ap()
    brep = nc.dram_tensor("brep", [R, 128, G], BF16, kind="ExternalInput").ap()
    gidx = nc.dram_tensor("gidx", [128, nsteps], I32, kind="ExternalInput").ap()
    out = nc.dram_tensor("out", [nloc, D], BF16, kind="ExternalOutput").ap()
    gx = nc.dram_tensor("gx", [nrows, G], BF16).ap()

    loc_bs = [nsteps - t for t in range(nsteps)]
    loc_off = np.concatenate([[0], np.cumsum(loc_bs)]).astype(int)

    with PatchedTileContext(nc) as tc:
        with tc.tile_pool(name="p1_xt", bufs=2) as xt_pool, \
             tc.tile_pool(name="p1_wf", bufs=8) as wf_pool, \
             tc.tile_pool(name="p1_bi", bufs=8) as bi_pool, \
             tc.tile_pool(name="p1_gx", bufs=1) as gxs_pool, \
             tc.tile_pool(name="p2_const", bufs=1) as const_pool, \
             tc.tile_pool(name="p2_gx", bufs=2) as gx_pool, \
             tc.tile_pool(name="p2_act", bufs=1) as act_pool, \
             tc.tile_pool(name="p2_st", bufs=1) as st_pool, \
             tc.tile_pool(name="p2_h", bufs=2) as h_pool, \
             tc.tile_pool(name="p2_ht", bufs=2) as ht_pool, \
             tc.tile_pool(name="p2_ps", bufs=4, space="PSUM") as ps_pool, \
             tc.tile_pool(name="p1_ps", bufs=1, space="PSUM") as p1ps_pool, \
             tc.tile_pool(name="p2_tr", bufs=1, space="PSUM") as tr_pool:

            # ---------------- phase-1 weights: all rels resident, loaded
            # lazily (first tile of each rel triggers the load) so prologue
            # tiles don't queue behind 16MB of weight DMA ------------------
            wf_sbs, bi_sbs = {}, {}

            def ensure_wf(r):
                if r not in wf_sbs:
                    wf_sb = wf_pool.tile([128, KD, G], BF16, tag="wf_sb")
                    nc.sync.dma_start(wf_sb[:], wf[r])
                    bi_sb = bi_pool.tile([128, G], BF16, tag="bi_sb")
                    nc.sync.dma_start(bi_sb[:], brep[r])
                    wf_sbs[r] = wf_sb
                    bi_sbs[r] = bi_sb

            # ---------------- phase-1 part emitter -----------------------
            # a part = one 1024-wide gate half of one tile (8 matmuls + add)
            p1_writes = []          # one DMA-write instr per tile, emit order
            p1_tile_state = {}
            emit_pos = [0]          # next part index (2 parts per tile)

            def emit_p1_part():
                pi = emit_pos[0]
                emit_pos[0] += 1
                e, jb = pi // 2, pi % 2
                i = emit_order[e]
                r = tile_rel[i]
                ensure_wf(r)
                if jb == 0:
                    xt_sb = xt_pool.tile([128, KD, 128], BF16, tag="xt_sb")
                    nc.sync.dma_start(xt_sb[:], xt[i])
                    gxs = gxs_pool.tile([128, G], BF16, tag="gxs")
                    p1_tile_state[e] = (xt_sb, gxs)
                xt_sb, gxs = p1_tile_state[e]
                sl = slice(jb * HW, (jb + 1) * HW)
                ps = p1ps_pool.tile([128, HW], F32, tag="p1ps")
                for k in range(KD):
                    for half in range(2):
                        hs = slice(half * 512, (half + 1) * 512)
                        ws = slice(jb * HW + half * 512,
                                   jb * HW + (half + 1) * 512)
                        nc.tensor.matmul(
                            ps[:, hs], xt_sb[:, k, :], wf_sbs[r][:, k, ws],
                            start=(k == 0), stop=(k == KD - 1),
                        )
                # bias add + cast to bf16 in one DVE pass
                nc.vector.tensor_add(gxs[:, sl], ps[:], bi_sbs[r][:, sl])
                if jb == 1:
                    wi = nc.sync.dma_start(gx[i * 128:(i + 1) * 128, :], gxs[:])
                    p1_writes.append(wi.ins)
                    del p1_tile_state[e]

            def ensure_written(n):
                while len(p1_writes) < min(n, ntiles):
                    emit_p1_part()

            def fill_to(n):
                target = 2 * min(n, ntiles)
                if emit_pos[0] < target:
                    emit_p1_part()

            # ---------------- phase 2: LSTM ------------------------------
            wh_sb = const_pool.tile([128, KD, G], BF16)
            nc.sync.dma_start(wh_sb[:], wh[:])
            idx_sb = const_pool.tile([128, nsteps], I32)
            nc.sync.dma_start(idx_sb[:], gidx[:])
            ident_b = const_pool.tile([128, 128], BF16)
            make_identity(nc, ident_b[:])

            c_sb = st_pool.tile([128, D], F32)
            tmp1 = st_pool.tile([128, D], F32)

            ht_sb = None
            gxt_tiles = {}
            banks = {}   # (t, jb) -> psum tile [128, 512]; jb = i,f,g,o

            def emit_gather(t):
                ensure_written(K[t])
                gxt = gx_pool.tile([128, G], BF16, tag="gxt")
                gi = nc.gpsimd.indirect_dma_start(
                    out=gxt[:],
                    out_offset=None,
                    in_=gx[0:nrows, :],
                    in_offset=bass.IndirectOffsetOnAxis(
                        ap=idx_sb[:, t:t + 1], axis=0
                    ),
                )
                # the tracker cannot see through the dynamic row offsets, so
                # order the gather after the writes it needs explicitly
                for w in p1_writes[:K[t]]:
                    add_dep_helper(gi.ins, w, reason="gather waits gx writes")
                gxt_tiles[t] = gxt

            def emit_ident(t, jb):
                # first write of gate bank jb for step t: gates <- gx rows
                psb = ps_pool.tile([128, 512], F32, tag="ps")
                nc.tensor.matmul(
                    psb[:], ident_b[:],
                    gxt_tiles[t][:, jb * 512:(jb + 1) * 512],
                    start=True, stop=(t == 0),
                )
                banks[(t, jb)] = psb

            # gate bank order: g first so the c-chain starts earliest
            BORD = (2, 0, 1, 3)   # g, i, f, o
            emit_gather(0)
            emit_gather(1)
            for jb in BORD:
                emit_ident(0, jb)
            for t in range(nsteps):
                bs = nsteps - t
                if t + 2 < nsteps:
                    emit_gather(t + 2)
                sif = act_pool.tile([128, 2 * D], F32, tag="sif")
                tg = act_pool.tile([128, D], F32, tag="tg")
                so = act_pool.tile([128, D], BF16, tag="so")

                def do_act_half(jb, hh, pop=False):
                    # half-granularity acts: the c-chain starts after only the
                    # first halves of sigmoid(g/i/f), not the full 512 columns
                    psb = banks.pop((t, jb)) if pop else banks[(t, jb)]
                    dst = {2: tg[:], 0: sif[:, 0:D], 1: sif[:, D:2 * D],
                           3: so[:]}[jb]
                    fn = AF.Tanh if jb == 2 else AF.Sigmoid
                    hs = slice(hh * (D // 2), (hh + 1) * (D // 2))
                    nc.scalar.activation(dst[:, hs], psb[:, hs], fn)

                # recurrent accumulation: consume hT half-by-half (k 0,1 then
                # 2,3) so it pipelines with the previous step's tail; within
                # each half k is outermost so consecutive matmuls share the
                # stationary operand.  Acts fire per bank after its k=3, but
                # the t+1 idents are deferred past the whole block so the k=3
                # tail is not serialized on the activations.
                if t > 0:
                    for ks in ((0, 1), (2, 3)):
                        for k in ks:
                            for jb in BORD:
                                nc.tensor.matmul(
                                    banks[(t, jb)][:],
                                    ht_sb[:, k * 128:(k + 1) * 128],
                                    wh_sb[:, k, jb * 512:(jb + 1) * 512],
                                    start=False,
                                    stop=(k == KD - 1),
                                )
                                if k == KD - 1:
                                    do_act_half(jb, 0)
                    for jb in BORD:
                        do_act_half(jb, 1, pop=True)
                    fill_to(K[min(t + 8, nsteps - 1)])
                else:
                    for jb in BORD:
                        do_act_half(jb, 0)
                        do_act_half(jb, 1, pop=True)
                if t + 1 < nsteps:
                    for jb in BORD:
                        emit_ident(t + 1, jb)
                fill_to(K[min(t + 8, nsteps - 1)])

                # c / h tail at half granularity so the next step's first
                # recurrent matmuls start as soon as half 0 is through
                h_sb = h_pool.tile([128, D], BF16, tag="h_sb")
                if t < nsteps - 1:
                    trp = tr_pool.tile([128, D], BF16, tag="trp")
                    new_ht = ht_pool.tile([128, D], BF16, tag="ht_sb")
                tc_sb = act_pool.tile([128, D], BF16, tag="tc_sb")
                H = D // 2
                for hh in range(2):
                    sl = slice(hh * H, (hh + 1) * H)
                    if t == 0:
                        nc.vector.tensor_tensor(
                            c_sb[:, sl], sif[:, sl], tg[:, sl],
                            mybir.AluOpType.mult,
                        )
                    else:
                        # i*g in place over the sigmoid(i) slice
                        nc.vector.tensor_tensor(
                            sif[:, sl], sif[:, sl], tg[:, sl],
                            mybir.AluOpType.mult,
                        )
                        nc.vector.tensor_tensor(
                            tmp1[:, sl], sif[:, D + hh * H:D + (hh + 1) * H],
                            c_sb[:, sl], mybir.AluOpType.mult,
                        )
                        nc.vector.tensor_add(c_sb[:, sl], tmp1[:, sl],
                                             sif[:, sl])
                    nc.scalar.activation(tc_sb[:, sl], c_sb[:, sl], AF.Tanh)
                    nc.vector.tensor_tensor(
                        h_sb[:, sl], so[:, sl], tc_sb[:, sl],
                        mybir.AluOpType.mult,
                    )
                    if t < nsteps - 1:
                        for k in (2 * hh, 2 * hh + 1):
                            nc.tensor.transpose(
                                trp[:, k * 128:(k + 1) * 128],
                                h_sb[:, k * 128:(k + 1) * 128],
                                ident_b[:],
                            )
                        nc.vector.tensor_copy(new_ht[:, sl], trp[:, sl])
                if t < nsteps - 1:
                    ht_sb = new_ht
                # stream out this step's hidden states (packed rows)
                nc.sync.dma_start(
                    out[int(loc_off[t]):int(loc_off[t]) + bs, :], h_sb[:bs, :]
                )
            ensure_written(ntiles)
    return nc


# ---------------------------------------------------------------------------
# Host-side data marshaling
# ---------------------------------------------------------------------------
def _expected_layout():
    lengths = T - np.arange(B) // NCORES
    batch_sizes = np.array([(lengths > t).sum() for t in range(T)], dtype=np.int32)
    time_idx = np.concatenate(
        [np.full(bs, t, np.int32) for t, bs in enumerate(batch_sizes)]
    )
    batch_idx = np.concatenate(
        [np.arange(bs, dtype=np.int32) for bs in batch_sizes]
    )
    return batch_sizes, time_idx, batch_idx


def _numpy_reference(embed, W_rel, b_rel, W_ih, W_hh, b_ih, b_hh,
                     nodes, rels, time_idx, batch_idx, batch_sizes):
    """Pure-numpy fallback (only used if the packed layout differs from the
    hardcoded one)."""
    n_steps = int(batch_sizes.shape[0])
    max_bs = int(batch_sizes.max())
    x = embed[nodes]
    y = np.zeros_like(x)
    for r in range(W_rel.shape[0]):
        m = rels == r
        y[m] = x[m] @ W_rel[r].T + b_rel[r]
    d = x.shape[-1]
    xp = np.zeros((n_steps, max_bs, d), x.dtype)
    mask = np.zeros((n_steps, max_bs), bool)
    xp[time_idx, batch_idx] = y
    mask[time_idx, batch_idx] = True
    bias = b_ih + b_hh

    def sig(v):
        return 1.0 / (1.0 + np.exp(-v))

    h = np.zeros((max_bs, d), x.dtype)
    c = np.zeros((max_bs, d), x.dtype)
    hs = np.zeros((n_steps, max_bs, d), x.dtype)
    for t in range(n_steps):
        gates = xp[t] @ W_ih.T + h @ W_hh.T + bias
        i, f, g, o = np.split(gates, 4, axis=-1)
        c_new = sig(f) * c + sig(i) * np.tanh(g)
        h_new = sig(o) * np.tanh(c_new)
        m = mask[t][:, None]
        h = np.where(m, h_new, h)
        c = np.where(m, c_new, c)
        hs[t] = h
    return hs[time_idx, batch_idx]


def _prepare_host(inputs, nsteps=T):
    """Build per-core device input dicts + the output unshard map."""
    embed = np.asarray(inputs["embed"], np.float32)
    W_rel = np.asarray(inputs["W_rel"], np.float32)
    b_rel = np.asarray(inputs["b_rel"], np.float32)
    W_ih = np.asarray(inputs["W_ih"], np.float32)
    W_hh = np.asarray(inputs["W_hh"], np.float32)
    b_ih = np.asarray(inputs["b_ih"], np.float32)
    b_hh = np.asarray(inputs["b_hh"], np.float32)
    nodes = np.asarray(inputs["nodes"])
    rels = np.asarray(inputs["rels"])

    nloc = nsteps * (nsteps + 1) // 2

    # fused weights & biases (float64 for accuracy, cast to bf16/f32)
    Wfuse = (W_ih.astype(np.float64) @ W_rel.astype(np.float64))
    Wfuse = Wfuse.astype(np.float32)            # [R, G, D]
    btot = (W_ih.astype(np.float64) @ b_rel.astype(np.float64).T).T \
        + (b_ih + b_hh).astype(np.float64)      # [R, G]
    btot = btot.astype(np.float32)

    wf_host = np.ascontiguousarray(
        Wfuse.transpose(0, 2, 1).reshape(R, KD, 128, G).transpose(0, 2, 1, 3)
    ).astype(BF16NP)                             # [R, 128(dk), KD, G]
    wh_host = np.ascontiguousarray(
        W_hh.T.reshape(KD, 128, G).transpose(1, 0, 2)
    ).astype(BF16NP)                             # [128(dk), KD, G]
    brep_host = np.ascontiguousarray(
        np.broadcast_to(btot[:, None, :], (R, 128, G))
    ).astype(BF16NP)

    # local token enumeration (identical structure for every core)
    t_arr = np.concatenate(
        [np.full(nsteps - t, t, np.int64) for t in range(nsteps)]
    )
    j_arr = np.concatenate(
        [np.arange(nsteps - t, dtype=np.int64) for t in range(nsteps)]
    )
    gbs = NCORES * (nsteps - np.arange(nsteps, dtype=np.int64))
    goff = np.concatenate([[0], np.cumsum(gbs)])

    # per-core per-rel token counts -> shared per-rel tile counts
    core_rel = []
    for core in range(NCORES):
        grow = goff[t_arr] + NCORES * j_arr + core
        rel_loc = rels[grow].astype(np.int64)
        core_rel.append(rel_loc)
    counts = np.array([
        np.bincount(core_rel[core], minlength=R) for core in range(NCORES)
    ])                                           # [NCORES, R]
    ntc = tuple(int(-(-counts[:, r].max() // 128)) for r in range(R))
    ntiles = sum(ntc)
    seg_base = np.concatenate([[0], np.cumsum(ntc)]) * 128  # per-rel row base

    # gather deadlines: n_r(t) = tiles of rel r needed by the step-t gather
    # (worst core); K[t] = total needed tiles; emit_order sorted by deadline.
    # max_cum[r, t] = max over cores of #{tokens of rel r with time <= t}
    max_cum = np.zeros((R, nsteps), np.int64)
    for core in range(NCORES):
        for r in range(R):
            sel = core_rel[core] == r
            cnt_t = np.bincount(t_arr[sel], minlength=nsteps)
            max_cum[r] = np.maximum(max_cum[r], np.cumsum(cnt_t))
    n_rt = -(-max_cum // 128)                    # [R, nsteps]
    K = n_rt.sum(axis=0).astype(int)             # [nsteps]
    tile_base = np.concatenate([[0], np.cumsum(ntc)])
    dl_list = []
    for r in range(R):
        for j in range(ntc[r]):
            need = np.nonzero(n_rt[r] > j)[0]
            dl = int(need[0]) if len(need) else nsteps - 1
            dl_list.append((dl, j, r, int(tile_base[r] + j)))
    dl_list.sort()
    emit_order = [phys for (_, _, _, phys) in dl_list]

    in_maps = []
    for core in range(NCORES):
        grow = goff[t_arr] + NCORES * j_arr + core
        node_loc = nodes[grow]
        rel_loc = core_rel[core]

        order = np.lexsort((j_arr, t_arr, rel_loc))
        # position within each rel segment
        cnt = np.bincount(rel_loc, minlength=R)
        q = np.concatenate([np.arange(c) for c in cnt])
        base_sorted = seg_base[rel_loc[order]]
        prow = np.empty(nloc, np.int64)
        prow[order] = base_sorted + q

        gidx_host = np.zeros((128, nsteps), np.int32)
        gidx_host[j_arr, t_arr] = prow

        Xp = np.zeros((ntiles * 128, D), np.float32)
        Xp[prow] = embed[node_loc]
        xt_host = np.ascontiguousarray(
            Xp.reshape(ntiles, 128, KD, 128).transpose(0, 3, 2, 1)
        ).astype(BF16NP)                         # [NT, 128(dk), KD, 128(tok)]

        in_maps.append({
            "xt": xt_host,
            "wf": wf_host,
            "wh": wh_host,
            "brep": brep_host,
            "gidx": gidx_host,
        })

    unshard = {
        "t_arr": t_arr, "j_arr": j_arr, "goff": goff,
        "nloc": nloc, "ntc": ntc, "emit_order": emit_order, "K": K,
    }
    return in_maps, unshard


def kernel(**inputs):
    global LAST_RESULTS
    import os

    # Verify the packed layout matches the hardcoded structure.
    bs_exp, ti_exp, bi_exp = _expected_layout()
    ok = (
        np.array_equal(np.asarray(inputs["batch_sizes"]), bs_exp)
        and np.array_equal(np.asarray(inputs["time_idx"]), ti_exp)
        and np.array_equal(np.asarray(inputs["batch_idx"]), bi_exp)
        and np.asarray(inputs["embed"]).shape == (50000, D)
    )
    if not ok:
        return _numpy_reference(**{k: np.asarray(v) for k, v in inputs.items()})

    in_maps, unshard = _prepare_host(inputs)

    nc = build_program(unshard["ntc"], unshard["emit_order"], unshard["K"])
    trace = bool(os.environ.get("KERNEL_TRACE"))
    res = bass_utils.run_bass_kernel_spmd(
        nc, in_maps, core_ids=list(range(NCORES)), trace=trace,
    )
    LAST_RESULTS = res

    t_arr = unshard["t_arr"]
    j_arr = unshard["j_arr"]
    goff = unshard["goff"]
    out_full = np.zeros((len(np.asarray(inputs["time_idx"])), D), np.float32)
    for core in range(NCORES):
        grow = goff[t_arr] + NCORES * j_arr + core
        out_full[grow] = np.asarray(res.results[core]["out"], np.float32)
    return out_full


# revision 40
# speedup vs baseline: 1.0281x; 1.0116x over previous
"""Trainium2 Bass kernel for nn_Evolution_4664334483942 (moe_routing).

Model: per-token relation-specific linear (MoE dispatch) feeding a packed
variable-length-sequence LSTM.

Strategy (data-parallel over sequences, 8 cores, no collectives):
  - Global batch b (0..1023) assigned to core b % 8.  Every core then holds
    128 sequences with lengths 128,127,...,1 (identical structure on every
    core), 8256 tokens each.
  - Host folds W_ih @ W_rel[r].T into per-relation fused weights so the MoE
    projection and the LSTM input projection collapse into ONE GEMM:
        gx[n] = x[n] @ Wfuse[rel_n].T + (W_ih b_rel[rel_n] + b_ih + b_hh)
  - All matmul operands are bf16 (stationary bf16 enables the PE fast
    weight load; moving bf16 allows 1024-wide streams spanning 2 PSUM
    banks), accumulation stays f32 in PSUM, the LSTM cell state stays f32.
  - Phase 1 (device): dense bf16 GEMM over rel-sorted 128-token tiles
    (per-rel tile counts sized to the worst core), writing gx (bf16) to
    DRAM.
  - Phase 2 (device): 128 sequential LSTM steps.  Each step gathers its
    gx rows via indirect DMA (per-core index table = data, so the SPMD
    instruction stream stays core-independent), injects them into the two
    1024-wide gate PSUM tiles via identity matmuls, accumulates h @ W_hh.T
    on top, applies sigmoid/tanh on ScalarE, c/h updates on VectorE,
    PE-transposes h (bf16) for the next step, and streams h out to DRAM.
"""

import numpy as np
import ml_dtypes

import concourse.bass as bass
import concourse.mybir as mybir
import concourse.tile as tile
from concourse import bass_utils
from concourse.masks import make_identity
from bass_rust import add_dep_helper
from concourse.vector_clock import ScopedClock

F32 = mybir.dt.float32
BF16 = mybir.dt.bfloat16
I32 = mybir.dt.int32
AF = mybir.ActivationFunctionType
BF16NP = ml_dtypes.bfloat16

NCORES = 8

# Problem constants (hardcoded; kernel.py must be self-contained).
D = 512          # hidden dim
R = 8            # relations
T = 128          # max sequence length / LSTM steps
B = 1024         # global sequences
KD = D // 128    # contraction k-tiles
G = 4 * D        # gate width (2048)
HW = 1024        # matmul moving-stream width (2 PSUM banks)

# Results of the last device run (test harness reads exec_time_ns from here).
LAST_RESULTS = None


# ---------------------------------------------------------------------------
# Walrus in this toolchain accepts only ONE sync-wait command per instruction;
# Tile's wait assignment can attach several.  Peel the extras onto same-engine
# NOPs placed immediately before the offending instruction.
# ---------------------------------------------------------------------------
def _split_waits_in_list(nc, insts, max_waits=1):
    out = []
    for inst in insts:
        si = inst.sync_info
        if si is not None and si.on_wait is not None and len(si.on_wait) > max_waits:
            waits = list(si.on_wait)
            for w in waits[max_waits:]:
                nop = mybir.InstNoOp(
                    name=nc.get_next_instruction_name(), ins=[], outs=[],
                )
                nop.engine = inst.engine
                nop.sync_info = mybir.SyncInfo(on_wait=[w], on_update=[])
                out.append(nop)
            inst.sync_info = mybir.SyncInfo(
                on_wait=waits[:max_waits], on_update=list(si.on_update or [])
            )
        out.append(inst)
    return out


class PatchedTileContext(tile.TileContext):
    def _lower_ordered_insts(self, ordered):
        for bb_name in list(ordered.keys()):
            ordered[bb_name] = _split_waits_in_list(self.nc, ordered[bb_name])
        super()._lower_ordered_insts(ordered)

    def _drain_and_barrier(self, tick_clock, wait_clock):
        nop_inst = self.nc.sync.nop()
        wait_clock.add_sem_waits(
            nop_inst.ins, ScopedClock({None: tick_clock.global_clock})
        )
        si = nop_inst.ins.sync_info
        if si is not None and si.on_wait and len(si.on_wait) > 1:
            waits = list(si.on_wait)
            nop_inst.ins.sync_info = mybir.SyncInfo(
                on_wait=[waits[0]], on_update=list(si.on_update or [])
            )
            for w in waits[1:]:
                extra = self.nc.sync.nop()
                extra.ins.sync_info = mybir.SyncInfo(on_wait=[w], on_update=[])
        self.nc.sync.drain()
        self.nc.all_engine_barrier()
        assert self.sems is not None
        popped = self.nc._tile_sem_poison_stack.pop()
        assert popped is self._sem_poison
        self.nc.clear_and_free_semaphores(list(self.sems.allocated().values()))
        self.nc.all_engine_barrier()


# ---------------------------------------------------------------------------
# Device program (core-independent instruction stream; per-core variation is
# carried entirely by input data: xt tile contents and the gather index table)
#
# ntc: tuple of R ints — tiles per relation (same on every core).
# emit_order: phase-1 tile emission order (physical tile indices), sorted by
#   gather deadline so tiles interleave into the LSTM's tensor-engine gaps.
# K: K[t] = number of tiles (prefix of emit_order) whose gx rows must be
#   written before the step-t gather may run (worst core).
# ---------------------------------------------------------------------------
def build_program(ntc, emit_order, K, nsteps=T):
    ntiles = sum(ntc)
    nrows = ntiles * 128
    nloc = nsteps * (nsteps + 1) // 2

    # physical tile order: rel-major
    tile_rel = []
    for r in range(R):
        tile_rel.extend([r] * ntc[r])

    nc = bass.Bass(target_bir_lowering=False, debug=False, trn_type="TRN2")

    xt = nc.dram_tensor("xt", [ntiles, 128, KD, 128], BF16, kind="ExternalInput").ap()
    wf = nc.dram_tensor("wf", [R, 128, KD, G], BF16, kind="ExternalInput").ap()
    wh = nc.dram_tensor("wh", [128, KD, G], BF16, kind="ExternalInput").ap()
    brep = nc.dram_tensor("brep", [R, 128, G], BF16, kind="ExternalInput").ap()
    gidx = nc.dram_tensor("gidx", [128, nsteps], I32, kind="ExternalInput").ap()
    out = nc.dram_tensor("out", [nloc, D], BF16, kind="ExternalOutput").ap()
    gx = nc.dram_tensor("gx", [nrows, G], BF16).ap()

    loc_bs = [nsteps - t for t in range(nsteps)]
    loc_off = np.concatenate([[0], np.cumsum(loc_bs)]).astype(int)

    with PatchedTileContext(nc) as tc:
        with tc.tile_pool(name="p1_xt", bufs=2) as xt_pool, \
             tc.tile_pool(name="p1_wf", bufs=8) as wf_pool, \
             tc.tile_pool(name="p1_bi", bufs=8) as bi_pool, \
             tc.tile_pool(name="p1_gx", bufs=1) as gxs_pool, \
             tc.tile_pool(name="p2_const", bufs=1) as const_pool, \
             tc.tile_pool(name="p2_gx", bufs=2) as gx_pool, \
             tc.tile_pool(name="p2_act", bufs=1) as act_pool, \
             tc.tile_pool(name="p2_st", bufs=1) as st_pool, \
             tc.tile_pool(name="p2_h", bufs=2) as h_pool, \
             tc.tile_pool(name="p2_ht", bufs=2) as ht_pool, \
             tc.tile_pool(name="p2_ps", bufs=4, space="PSUM") as ps_pool, \
             tc.tile_pool(name="p1_ps", bufs=1, space="PSUM") as p1ps_pool, \
             tc.tile_pool(name="p2_tr", bufs=1, space="PSUM") as tr_pool, \
             tc.tile_pool(name="p2_dm", bufs=1, space="PSUM") as dm_pool:

            # ---------------- phase-1 weights: all rels resident, loaded
            # lazily (first tile of each rel triggers the load) so prologue
            # tiles don't queue behind 16MB of weight DMA ------------------
            wf_sbs, bi_sbs = {}, {}

            def ensure_wf(r):
                if r not in wf_sbs:
                    wf_sb = wf_pool.tile([128, KD, G], BF16, tag="wf_sb")
                    nc.sync.dma_start(wf_sb[:], wf[r])
                    bi_sb = bi_pool.tile([128, G], BF16, tag="bi_sb")
                    nc.sync.dma_start(bi_sb[:], brep[r])
                    wf_sbs[r] = wf_sb
                    bi_sbs[r] = bi_sb

            # ---------------- phase-1 part emitter -----------------------
            # a part = one 1024-wide gate half of one tile (8 matmuls + add)
            p1_writes = []          # one DMA-write instr per tile, emit order
            p1_tile_state = {}
            emit_pos = [0]          # next part index (2 parts per tile)

            def emit_p1_part():
                pi = emit_pos[0]
                emit_pos[0] += 1
                e, jb = pi // 2, pi % 2
                i = emit_order[e]
                r = tile_rel[i]
                ensure_wf(r)
                if jb == 0:
                    xt_sb = xt_pool.tile([128, KD, 128], BF16, tag="xt_sb")
                    nc.sync.dma_start(xt_sb[:], xt[i])
                    gxs = gxs_pool.tile([128, G], BF16, tag="gxs")
                    p1_tile_state[e] = (xt_sb, gxs)
                xt_sb, gxs = p1_tile_state[e]
                sl = slice(jb * HW, (jb + 1) * HW)
                ps = p1ps_pool.tile([128, HW], F32, tag="p1ps")
                for k in range(KD):
                    for half in range(2):
                        hs = slice(half * 512, (half + 1) * 512)
                        ws = slice(jb * HW + half * 512,
                                   jb * HW + (half + 1) * 512)
                        nc.tensor.matmul(
                            ps[:, hs], xt_sb[:, k, :], wf_sbs[r][:, k, ws],
                            start=(k == 0), stop=(k == KD - 1),
                        )
                # bias add + cast to bf16 in one DVE pass
                nc.vector.tensor_add(gxs[:, sl], ps[:], bi_sbs[r][:, sl])
                if jb == 1:
                    wi = nc.sync.dma_start(gx[i * 128:(i + 1) * 128, :], gxs[:])
                    p1_writes.append(wi.ins)
                    del p1_tile_state[e]

            def ensure_written(n):
                while len(p1_writes) < min(n, ntiles):
                    emit_p1_part()

            def fill_to(n):
                target = 2 * min(n, ntiles)
                if emit_pos[0] < target:
                    emit_p1_part()

            # ---------------- phase 2: LSTM ------------------------------
            wh_sb = const_pool.tile([128, KD, G], BF16)
            nc.sync.dma_start(wh_sb[:], wh[:])
            idx_sb = const_pool.tile([128, nsteps], I32)
            nc.sync.dma_start(idx_sb[:], gidx[:])
            ident_b = const_pool.tile([128, 128], BF16)
            make_identity(nc, ident_b[:])

            c_sb = st_pool.tile([128, D], F32)
            tmp1 = st_pool.tile([128, D], F32)

            ht_sb = None
            gxt_tiles = {}
            banks = {}   # (t, jb) -> psum tile [128, 512]; jb = i,f,g,o

            def emit_gather(t):
                ensure_written(K[t])
                gxt = gx_pool.tile([128, G], BF16, tag="gxt")
                gi = nc.gpsimd.indirect_dma_start(
                    out=gxt[:],
                    out_offset=None,
                    in_=gx[0:nrows, :],
                    in_offset=bass.IndirectOffsetOnAxis(
                        ap=idx_sb[:, t:t + 1], axis=0
                    ),
                )
                # the tracker cannot see through the dynamic row offsets, so
                # order the gather after the writes it needs explicitly
                for w in p1_writes[:K[t]]:
                    add_dep_helper(gi.ins, w, reason="gather waits gx writes")
                gxt_tiles[t] = gxt

            def emit_ident(t, jb):
                # first write of gate bank jb for step t: gates <- gx rows
                psb = ps_pool.tile([128, 512], F32, tag="ps")
                nc.tensor.matmul(
                    psb[:], ident_b[:],
                    gxt_tiles[t][:, jb * 512:(jb + 1) * 512],
                    start=True, stop=(t == 0),
                )
                banks[(t, jb)] = psb

            # gate bank order: g first so the c-chain starts earliest
            BORD = (2, 0, 1, 3)   # g, i, f, o
            emit_gather(0)
            emit_gather(1)
            for jb in BORD:
                emit_ident(0, jb)
            for t in range(nsteps):
                bs = nsteps - t
                if t + 2 < nsteps:
                    emit_gather(t + 2)
                sif = act_pool.tile([128, 2 * D], F32, tag="sif")
                tg = act_pool.tile([128, D], F32, tag="tg")
                so = act_pool.tile([128, D], BF16, tag="so")

                def do_act_half(jb, hh, pop=False):
                    # half-granularity acts: the c-chain starts after only the
                    # first halves of sigmoid(g/i/f), not the full 512 columns
                    psb = banks.pop((t, jb)) if pop else banks[(t, jb)]
                    dst = {2: tg[:], 0: sif[:, 0:D], 1: sif[:, D:2 * D],
                           3: so[:]}[jb]
                    fn = AF.Tanh if jb == 2 else AF.Sigmoid
                    hs = slice(hh * (D // 2), (hh + 1) * (D // 2))
                    nc.scalar.activation(dst[:, hs], psb[:, hs], fn)

                # recurrent accumulation: consume hT half-by-half (k 0,1 then
                # 2,3) so it pipelines with the previous step's tail; within
                # each half k is outermost so consecutive matmuls share the
                # stationary operand.  Acts fire per bank after its k=3, but
                # the t+1 idents are deferred past the whole block so the k=3
                # tail is not serialized on the activations.
                pos0 = emit_pos[0]
                if t > 0:
                    for ks in ((0, 1), (2, 3)):
                        for k in ks:
                            for jb in BORD:
                                nc.tensor.matmul(
                                    banks[(t, jb)][:],
                                    ht_sb[:, k * 128:(k + 1) * 128],
                                    wh_sb[:, k, jb * 512:(jb + 1) * 512],
                                    start=False,
                                    stop=(k == KD - 1),
                                )
                                if k == KD - 1:
                                    do_act_half(jb, 0)
                    for jb in BORD:
                        do_act_half(jb, 1, pop=True)
                    fill_to(K[min(t + 8, nsteps - 1)])
                else:
                    for jb in BORD:
                        do_act_half(jb, 0)
                        do_act_half(jb, 1, pop=True)
                if t + 1 < nsteps:
                    for jb in BORD:
                        emit_ident(t + 1, jb)
                fill_to(K[min(t + 8, nsteps - 1)])
                # pad the window between the recurrence and the h transposes
                # to a constant amount of tensor-engine work: steps that drew
                # no phase-1 fill would otherwise idle here, cooling the PE
                # clock gate and making the next burst run at half rate.
                # Dummies are dependency-free matmuls into the spare bank.
                n_dummy = max(0, 10 - 8 * (emit_pos[0] - pos0) - 4)
                for _ in range(n_dummy):
                    dps = dm_pool.tile([128, 512], F32, tag="dummy")
                    nc.tensor.matmul(
                        dps[:], ident_b[:], wh_sb[:, 0, 0:512],
                        start=True, stop=True,
                    )

                # c / h tail at half granularity so the next step's first
                # recurrent matmuls start as soon as half 0 is through
                h_sb = h_pool.tile([128, D], BF16, tag="h_sb")
                if t < nsteps - 1:
                    trp = tr_pool.tile([128, D], BF16, tag="trp")
                    new_ht = ht_pool.tile([128, D], BF16, tag="ht_sb")
                tc_sb = act_pool.tile([128, D], BF16, tag="tc_sb")
                H = D // 2
                for hh in range(2):
                    sl = slice(hh * H, (hh + 1) * H)
                    if t == 0:
                        nc.vector.tensor_tensor(
                            c_sb[:, sl], sif[:, sl], tg[:, sl],
                            mybir.AluOpType.mult,
                        )
                    else:
                        # i*g in place over the sigmoid(i) slice
                        nc.vector.tensor_tensor(
                            sif[:, sl], sif[:, sl], tg[:, sl],
                            mybir.AluOpType.mult,
                        )
                        nc.vector.tensor_tensor(
                            tmp1[:, sl], sif[:, D + hh * H:D + (hh + 1) * H],
                            c_sb[:, sl], mybir.AluOpType.mult,
                        )
                        nc.vector.tensor_add(c_sb[:, sl], tmp1[:, sl],
                                             sif[:, sl])
                    nc.scalar.activation(tc_sb[:, sl], c_sb[:, sl], AF.Tanh)
                    nc.vector.tensor_tensor(
                        h_sb[:, sl], so[:, sl], tc_sb[:, sl],
                        mybir.AluOpType.mult,
                    )
                    if t < nsteps - 1:
                        for k in (2 * hh, 2 * hh + 1):
                            nc.tensor.transpose(
                                trp[:, k * 128:(k + 1) * 128],
                                h_sb[:, k * 128:(k + 1) * 128],
                                ident_b[:],
                            )
                        nc.vector.tensor_copy(new_ht[:, sl], trp[:, sl])
                if t < nsteps - 1:
                    ht_sb = new_ht
                # stream out this step's hidden states (packed rows)
                nc.sync.dma_start(
                    out[int(loc_off[t]):int(loc_off[t]) + bs, :], h_sb[:bs, :]
                )
            ensure_written(ntiles)
    return nc


# ---------------------------------------------------------------------------
# Host-side data marshaling
# ---------------------------------------------------------------------------
def _expected_layout():
    lengths = T - np.arange(B) // NCORES
    batch_sizes = np.array([(lengths > t).sum() for t in range(T)], dtype=np.int32)
    time_idx = np.concatenate(
        [np.full(bs, t, np.int32) for t, bs in enumerate(batch_sizes)]
    )
    batch_idx = np.concatenate(
        [np.arange(bs, dtype=np.int32) for bs in batch_sizes]
    )
    return batch_sizes, time_idx, batch_idx


def _numpy_reference(embed, W_rel, b_rel, W_ih, W_hh, b_ih, b_hh,
                     nodes, rels, time_idx, batch_idx, batch_sizes):
    """Pure-numpy fallback (only used if the packed layout differs from the
    hardcoded one)."""
    n_steps = int(batch_sizes.shape[0])
    max_bs = int(batch_sizes.max())
    x = embed[nodes]
    y = np.zeros_like(x)
    for r in range(W_rel.shape[0]):
        m = rels == r
        y[m] = x[m] @ W_rel[r].T + b_rel[r]
    d = x.shape[-1]
    xp = np.zeros((n_steps, max_bs, d), x.dtype)
    mask = np.zeros((n_steps, max_bs), bool)
    xp[time_idx, batch_idx] = y
    mask[time_idx, batch_idx] = True
    bias = b_ih + b_hh

    def sig(v):
        return 1.0 / (1.0 + np.exp(-v))

    h = np.zeros((max_bs, d), x.dtype)
    c = np.zeros((max_bs, d), x.dtype)
    hs = np.zeros((n_steps, max_bs, d), x.dtype)
    for t in range(n_steps):
        gates = xp[t] @ W_ih.T + h @ W_hh.T + bias
        i, f, g, o = np.split(gates, 4, axis=-1)
        c_new = sig(f) * c + sig(i) * np.tanh(g)
        h_new = sig(o) * np.tanh(c_new)
        m = mask[t][:, None]
        h = np.where(m, h_new, h)
        c = np.where(m, c_new, c)
        hs[t] = h
    return hs[time_idx, batch_idx]


def _prepare_host(inputs, nsteps=T):
    """Build per-core device input dicts + the output unshard map."""
    embed = np.asarray(inputs["embed"], np.float32)
    W_rel = np.asarray(inputs["W_rel"], np.float32)
    b_rel = np.asarray(inputs["b_rel"], np.float32)
    W_ih = np.asarray(inputs["W_ih"], np.float32)
    W_hh = np.asarray(inputs["W_hh"], np.float32)
    b_ih = np.asarray(inputs["b_ih"], np.float32)
    b_hh = np.asarray(inputs["b_hh"], np.float32)
    nodes = np.asarray(inputs["nodes"])
    rels = np.asarray(inputs["rels"])

    nloc = nsteps * (nsteps + 1) // 2

    # fused weights & biases (float64 for accuracy, cast to bf16/f32)
    Wfuse = (W_ih.astype(np.float64) @ W_rel.astype(np.float64))
    Wfuse = Wfuse.astype(np.float32)            # [R, G, D]
    btot = (W_ih.astype(np.float64) @ b_rel.astype(np.float64).T).T \
        + (b_ih + b_hh).astype(np.float64)      # [R, G]
    btot = btot.astype(np.float32)

    wf_host = np.ascontiguousarray(
        Wfuse.transpose(0, 2, 1).reshape(R, KD, 128, G).transpose(0, 2, 1, 3)
    ).astype(BF16NP)                             # [R, 128(dk), KD, G]
    wh_host = np.ascontiguousarray(
        W_hh.T.reshape(KD, 128, G).transpose(1, 0, 2)
    ).astype(BF16NP)                             # [128(dk), KD, G]
    brep_host = np.ascontiguousarray(
        np.broadcast_to(btot[:, None, :], (R, 128, G))
    ).astype(BF16NP)

    # local token enumeration (identical structure for every core)
    t_arr = np.concatenate(
        [np.full(nsteps - t, t, np.int64) for t in range(nsteps)]
    )
    j_arr = np.concatenate(
        [np.arange(nsteps - t, dtype=np.int64) for t in range(nsteps)]
    )
    gbs = NCORES * (nsteps - np.arange(nsteps, dtype=np.int64))
    goff = np.concatenate([[0], np.cumsum(gbs)])

    # per-core per-rel token counts -> shared per-rel tile counts
    core_rel = []
    for core in range(NCORES):
        grow = goff[t_arr] + NCORES * j_arr + core
        rel_loc = rels[grow].astype(np.int64)
        core_rel.append(rel_loc)
    counts = np.array([
        np.bincount(core_rel[core], minlength=R) for core in range(NCORES)
    ])                                           # [NCORES, R]
    ntc = tuple(int(-(-counts[:, r].max() // 128)) for r in range(R))
    ntiles = sum(ntc)
    seg_base = np.concatenate([[0], np.cumsum(ntc)]) * 128  # per-rel row base

    # gather deadlines: n_r(t) = tiles of rel r needed by the step-t gather
    # (worst core); K[t] = total needed tiles; emit_order sorted by deadline.
    # max_cum[r, t] = max over cores of #{tokens of rel r with time <= t}
    max_cum = np.zeros((R, nsteps), np.int64)
    for core in range(NCORES):
        for r in range(R):
            sel = core_rel[core] == r
            cnt_t = np.bincount(t_arr[sel], minlength=nsteps)
            max_cum[r] = np.maximum(max_cum[r], np.cumsum(cnt_t))
    n_rt = -(-max_cum // 128)                    # [R, nsteps]
    K = n_rt.sum(axis=0).astype(int)             # [nsteps]
    tile_base = np.concatenate([[0], np.cumsum(ntc)])
    dl_list = []
    for r in range(R):
        for j in range(ntc[r]):
            need = np.nonzero(n_rt[r] > j)[0]
            dl = int(need[0]) if len(need) else nsteps - 1
            dl_list.append((dl, j, r, int(tile_base[r] + j)))
    dl_list.sort()
    emit_order = [phys for (_, _, _, phys) in dl_list]

    in_maps = []
    for core in range(NCORES):
        grow = goff[t_arr] + NCORES * j_arr + core
        node_loc = nodes[grow]
        rel_loc = core_rel[core]

        order = np.lexsort((j_arr, t_arr, rel_loc))
        # position within each rel segment
        cnt = np.bincount(rel_loc, minlength=R)
        q = np.concatenate([np.arange(c) for c in cnt])
        base_sorted = seg_base[rel_loc[order]]
        prow = np.empty(nloc, np.int64)
        prow[order] = base_sorted + q

        gidx_host = np.zeros((128, nsteps), np.int32)
        gidx_host[j_arr, t_arr] = prow

        Xp = np.zeros((ntiles * 128, D), np.float32)
        Xp[prow] = embed[node_loc]
        xt_host = np.ascontiguousarray(
            Xp.reshape(ntiles, 128, KD, 128).transpose(0, 3, 2, 1)
        ).astype(BF16NP)                         # [NT, 128(dk), KD, 128(tok)]

        in_maps.append({
            "xt": xt_host,
            "wf": wf_host,
            "wh": wh_host,
            "brep": brep_host,
            "gidx": gidx_host,
        })

    unshard = {
        "t_arr": t_arr, "j_arr": j_arr, "goff": goff,
        "nloc": nloc, "ntc": ntc, "emit_order": emit_order, "K": K,
    }
    return in_maps, unshard


def kernel(**inputs):
    global LAST_RESULTS
    import os

    # Verify the packed layout matches the hardcoded structure.
    bs_exp, ti_exp, bi_exp = _expected_layout()
    ok = (
        np.array_equal(np.asarray(inputs["batch_sizes"]), bs_exp)
        and np.array_equal(np.asarray(inputs["time_idx"]), ti_exp)
        and np.array_equal(np.asarray(inputs["batch_idx"]), bi_exp)
        and np.asarray(inputs["embed"]).shape == (50000, D)
    )
    if not ok:
        return _numpy_reference(**{k: np.asarray(v) for k, v in inputs.items()})

    in_maps, unshard = _prepare_host(inputs)

    nc = build_program(unshard["ntc"], unshard["emit_order"], unshard["K"])
    trace = bool(os.environ.get("KERNEL_TRACE"))
    res = bass_utils.run_bass_kernel_spmd(
        nc, in_maps, core_ids=list(range(NCORES)), trace=trace,
    )
    LAST_RESULTS = res

    t_arr = unshard["t_arr"]
    j_arr = unshard["j_arr"]
    goff = unshard["goff"]
    out_full = np.zeros((len(np.asarray(inputs["time_idx"])), D), np.float32)
    for core in range(NCORES):
        grow = goff[t_arr] + NCORES * j_arr + core
        out_full[grow] = np.asarray(res.results[core]["out"], np.float32)
    return out_full
